# Initial kernel scaffold
#
"""Batched Kalman-gain kernel for Trainium2 (Bass/Tile), 8-core data parallel.

Per batch b (262144 of them):
    Sigma = F Sp F^T + Q            [8,8]
    S     = H Sigma H^T + R         [4,4]
    KG    = Sigma H^T S^-1          [8,4]

Factored to avoid materializing Sigma:
    A   = H F                       [4,8]
    C   = Sp A^T                    [8,4]
    P12 = F C + Q H^T  (= Sigma H^T) [8,4]
    S   = H P12 + R                 [4,4]
    X   = S^-1  (SPD, 2x2-block Schur complement)
    KG  = P12 X

Mapping: "planes" layout. 128 SBUF partitions = batch lanes; each lane holds
G consecutive batches' matrices along the free axis. Every per-batch product
is a wide elementwise tensor_tensor with broadcast access patterns
(DVE/GPSIMD). All contraction *sums* ride the TensorEngine for free: an
identity stationary operand (bitcast float32r -> 1 cycle/row) turns PSUM
accumulation across matmuls into elementwise tile summation. ScalarE (ACT)
evacuates PSUM->SBUF. The 4x4 SPD inverse is elementwise via the Schur
complement of the leading 2x2 block.

The per-chunk stages are software-pipelined with skew (engine queues are
FIFO: without skew, each engine stalls on the intra-chunk chain
A -> C -> P12 -> S -> X -> KG):
    iter t:  load(t) | A(t-1) | C,P12(t-2) | KG(t-4) | S,inv(t-3)
"""

import os

import numpy as np

P = 128          # SBUF partitions (batch lanes)
G = 32           # consecutive batches per lane per chunk
B = 262144       # full problem batch
NCORES = 8
B_CORE = B // NCORES           # 32768 per core
CHUNK = P * G                  # 4096 batches per chunk
NCHUNK = B_CORE // CHUNK       # 8 chunks

_NC_CACHE = {}


def _build_nc(b_core=B_CORE, g=G, repeat=1):
    import concourse.bacc as bacc
    import concourse.mybir as mybir
    import concourse.tile as tile
    from concourse.masks import make_identity

    fp32 = mybir.dt.float32
    fp32r = mybir.dt.float32r
    MULT = mybir.AluOpType.mult

    nchunk = b_core // (P * g)
    assert nchunk * P * g == b_core
    nc = bacc.Bacc("TRN2", target_bir_lowering=False, debug=False)

    F_d = nc.dram_tensor("F", [b_core, 8, 8], fp32, kind="ExternalInput").ap()
    H_d = nc.dram_tensor("H", [b_core, 4, 8], fp32, kind="ExternalInput").ap()
    Sp_d = nc.dram_tensor(
        "Sigma_previous", [b_core, 8, 8], fp32, kind="ExternalInput"
    ).ap()
    Q_d = nc.dram_tensor("Q", [b_core, 8, 8], fp32, kind="ExternalInput").ap()
    R_d = nc.dram_tensor("R", [b_core, 4, 4], fp32, kind="ExternalInput").ap()
    KG_d = nc.dram_tensor("KG", [b_core, 8, 4], fp32, kind="ExternalOutput").ap()

    # chunk views: batch = c*(P*g) + p*g + g_idx  (lane-contiguous DMA)
    Fv = F_d.rearrange("(c p g) i j -> c p g i j", p=P, g=g)
    Hv = H_d.rearrange("(c p g) m j -> c p g m j", p=P, g=g)
    Spv = Sp_d.rearrange("(c p g) i j -> c p g i j", p=P, g=g)
    Qv = Q_d.rearrange("(c p g) i j -> c p g i j", p=P, g=g)
    Rv = R_d.rearrange("(c p g) m n -> c p g m n", p=P, g=g)
    KGv = KG_d.rearrange("(c p g) i m -> c p g i m", p=P, g=g)

    BANK = 512  # fp32 elems per PSUM bank per partition
    all_dve = os.environ.get("ALL_DVE", "0") == "1"

    with tile.TileContext(nc) as tc:
        with (
            tc.tile_pool(name="consts", bufs=1) as consts,
            tc.tile_pool(name="ins3", bufs=3) as insp,
            tc.tile_pool(name="ins4", bufs=4) as insp2,
            tc.tile_pool(name="mid2", bufs=2) as midp,
            tc.tile_pool(name="mid3", bufs=3) as midp3,
            tc.tile_pool(name="prod", bufs=4) as prodp,
            tc.tile_pool(name="inv", bufs=2) as invp,
            tc.tile_pool(name="psum", bufs=8, space="PSUM") as psump,
        ):
            ident = consts.tile([P, P], fp32, tag="ident")
            make_identity(nc, ident[:])
            identr_t = consts.tile([P, P], fp32r, tag="identr")
            nc.vector.tensor_copy(identr_t[:], ident[:])
            identr = identr_t[:]

            def flat(t):
                return t[:].rearrange("p g a b -> p (g a b)")

            def contract(terms, out_tag, width, pool, extra_rhs=None):
                """terms: list of (engine, in0_ap, in1_ap, prod_shape[2:]).
                Returns SBUF tile [P, g, a, b] = sum of products (+extra_rhs).
                Products are elementwise TT ops; the sum runs on the PE via
                float32r identity matmuls accumulating in PSUM."""
                npc = (width + BANK - 1) // BANK
                rhs_list = []
                for eng, a_ap, b_ap, (d0, d1) in terms:
                    etag = "prodv" if eng is nc.vector else "prodg"
                    pt = prodp.tile([P, g, d0, d1], fp32r, tag=etag, name=etag)
                    eng.tensor_tensor(pt[:], a_ap, b_ap, op=MULT)
                    rhs_list.append(flat(pt))
                if extra_rhs is not None:
                    rhs_list.append(extra_rhs)
                d0, d1 = terms[0][3]
                out = pool.tile([P, g, d0, d1], fp32, tag=out_tag, name=out_tag)
                outf = flat(out)
                ps_tiles = [
                    psump.tile([P, BANK], fp32, tag="ps", name=f"ps_{out_tag}_{pc}")
                    for pc in range(npc)
                ]
                nterm = len(rhs_list)
                for pc in range(npc):
                    lo, hi = pc * BANK, min((pc + 1) * BANK, width)
                    for t, rhs in enumerate(rhs_list):
                        nc.tensor.matmul(
                            ps_tiles[pc][:, : hi - lo],
                            identr,
                            rhs[:, lo:hi],
                            start=(t == 0),
                            stop=(t == nterm - 1),
                        )
                    nc.scalar.copy(outf[:, lo:hi], ps_tiles[pc][:, : hi - lo])
                return out

            def bc(ap, axis, shape):
                return ap.unsqueeze(axis).broadcast_to(shape)

            st = [dict() for _ in range(nchunk)]
            V = nc.vector
            GP = nc.vector if all_dve else nc.gpsimd
            sh48 = [P, g, 4, 8]
            sh84 = [P, g, 8, 4]
            sh44 = [P, g, 4, 4]
            sh22 = [P, g, 2, 2]

            def emit_load(c):
                s = st[c]
                s["F"] = insp.tile([P, g, 8, 8], fp32, tag="F", name="Ft")
                s["Sp"] = insp.tile([P, g, 8, 8], fp32, tag="Sp", name="Spt")
                s["Q"] = insp.tile([P, g, 8, 8], fp32, tag="Q", name="Qt")
                s["H"] = insp2.tile([P, g, 4, 8], fp32, tag="H", name="Ht")
                s["R"] = insp2.tile([P, g, 4, 4], fp32, tag="R", name="Rt")
                nc.sync.dma_start(out=s["F"][:], in_=Fv[c])
                nc.sync.dma_start(out=s["H"][:], in_=Hv[c])
                nc.sync.dma_start(out=s["Sp"][:], in_=Spv[c])
                nc.sync.dma_start(out=s["Q"][:], in_=Qv[c])
                nc.sync.dma_start(out=s["R"][:], in_=Rv[c])

            def emit_A(c):
                s = st[c]
                Ft, Ht = s["F"], s["H"]
                s["A"] = contract(
                    [
                        (V, bc(Ht[:, :, :, j], 3, sh48), bc(Ft[:, :, j, :], 2, sh48), (4, 8))
                        for j in range(8)
                    ],
                    "A",
                    g * 32,
                    midp,
                )

            def emit_CP(c):
                s = st[c]
                Ft, Spt, Qt, Ht, A = s["F"], s["Sp"], s["Q"], s["H"], s["A"]
                C = contract(
                    [
                        (V, bc(Spt[:, :, :, k], 3, sh84), bc(A[:, :, :, k], 2, sh84), (8, 4))
                        for k in range(8)
                    ],
                    "C",
                    g * 32,
                    midp,
                )
                # Q*H^T terms first: they don't depend on C, so they fill the
                # engine while C is still accumulating/evacuating.
                s["P12"] = contract(
                    [
                        (GP, bc(Qt[:, :, :, j], 3, sh84), bc(Ht[:, :, :, j], 2, sh84), (8, 4))
                        for j in range(8)
                    ]
                    + [
                        (V, bc(Ft[:, :, :, j], 3, sh84), bc(C[:, :, j, :], 2, sh84), (8, 4))
                        for j in range(8)
                    ],
                    "P12",
                    g * 32,
                    midp3,
                )

            def emit_SX(c):
                s = st[c]
                Ht, Rt, P12 = s["H"], s["R"], s["P12"]
                Rr = prodp.tile([P, g, 4, 4], fp32r, tag="prodg", name="Rr")
                GP.tensor_copy(Rr[:], Rt[:])
                S = contract(
                    [
                        (GP, bc(Ht[:, :, :, i], 3, sh44), bc(P12[:, :, i, :], 2, sh44), (4, 4))
                        for i in range(8)
                    ],
                    "S",
                    g * 16,
                    midp,
                    extra_rhs=flat(Rr),
                )
                # ---- X = S^-1 via Schur complement of leading 2x2 block ----
                X = midp.tile([P, g, 4, 4], fp32, tag="X", name="X")
                Pi = invp.tile([P, g, 2, 2], fp32, tag="Pi", name="Pi")
                W = invp.tile([P, g, 2, 2], fp32, tag="W", name="W")
                Sc = invp.tile([P, g, 2, 2], fp32, tag="Sc", name="Sc")
                t3 = invp.tile([P, g, 2, 2], fp32, tag="t3", name="t3")
                t4 = invp.tile([P, g, 2, 2], fp32, tag="t4", name="t4")
                t5 = invp.tile([P, g, 2, 2], fp32, tag="t5", name="t5")
                t7 = invp.tile([P, g, 2, 2], fp32, tag="t7", name="t7")
                d0 = invp.tile([P, g], fp32, tag="d0", name="d0")
                u0 = invp.tile([P, g], fp32, tag="u0", name="u0")
                u1 = invp.tile([P, g], fp32, tag="u1", name="u1")
                r0 = invp.tile([P, g], fp32, tag="r0", name="r0")
                nr0 = invp.tile([P, g], fp32, tag="nr0", name="nr0")
                d1 = invp.tile([P, g], fp32, tag="d1", name="d1")
                r1 = invp.tile([P, g], fp32, tag="r1", name="r1")
                nr1 = invp.tile([P, g], fp32, tag="nr1", name="nr1")

                sa = S[:, :, 0, 0]
                sb = S[:, :, 0, 1]
                sb2 = S[:, :, 1, 0]
                sc_ = S[:, :, 1, 1]
                V.tensor_mul(u0[:], sa, sc_)
                V.tensor_mul(u1[:], sb, sb2)
                V.tensor_sub(d0[:], u0[:], u1[:])
                V.reciprocal(r0[:], d0[:])
                V.tensor_scalar_mul(nr0[:], r0[:], -1.0)
                V.tensor_mul(Pi[:, :, 0, 0], sc_, r0[:])
                V.tensor_mul(Pi[:, :, 1, 1], sa, r0[:])
                V.tensor_mul(Pi[:, :, 0, 1], sb, nr0[:])
                V.tensor_mul(Pi[:, :, 1, 0], sb2, nr0[:])
                # W = B Aq^-1
                V.tensor_mul(
                    W[:], bc(S[:, :, 2:4, 0], 3, sh22), bc(Pi[:, :, 0, :], 2, sh22)
                )
                V.tensor_mul(
                    t4[:], bc(S[:, :, 2:4, 1], 3, sh22), bc(Pi[:, :, 1, :], 2, sh22)
                )
                V.tensor_add(W[:], W[:], t4[:])
                # Sc = D - W B^T
                V.tensor_mul(
                    t3[:], bc(W[:, :, :, 0], 3, sh22), bc(S[:, :, 2:4, 0], 2, sh22)
                )
                V.tensor_mul(
                    t4[:], bc(W[:, :, :, 1], 3, sh22), bc(S[:, :, 2:4, 1], 2, sh22)
                )
                V.tensor_add(t3[:], t3[:], t4[:])
                V.tensor_sub(Sc[:], S[:, :, 2:4, 2:4], t3[:])
                # Sc^-1 -> X[2:4,2:4]
                V.tensor_mul(u0[:], Sc[:, :, 0, 0], Sc[:, :, 1, 1])
                V.tensor_mul(u1[:], Sc[:, :, 0, 1], Sc[:, :, 1, 0])
                V.tensor_sub(d1[:], u0[:], u1[:])
                V.reciprocal(r1[:], d1[:])
                V.tensor_scalar_mul(nr1[:], r1[:], -1.0)
                V.tensor_mul(X[:, :, 2, 2], Sc[:, :, 1, 1], r1[:])
                V.tensor_mul(X[:, :, 3, 3], Sc[:, :, 0, 0], r1[:])
                V.tensor_mul(X[:, :, 2, 3], Sc[:, :, 0, 1], nr1[:])
                V.tensor_mul(X[:, :, 3, 2], Sc[:, :, 1, 0], nr1[:])
                # X21 = -Sc^-1 W -> X[2:4,0:2]
                V.tensor_mul(
                    t5[:], bc(X[:, :, 2:4, 2], 3, sh22), bc(W[:, :, 0, :], 2, sh22)
                )
                V.tensor_mul(
                    t4[:], bc(X[:, :, 2:4, 3], 3, sh22), bc(W[:, :, 1, :], 2, sh22)
                )
                V.tensor_add(t5[:], t5[:], t4[:])
                V.tensor_scalar_mul(X[:, :, 2:4, 0:2], t5[:], -1.0)
                # X12 = X21^T
                V.tensor_copy(
                    X[:, :, 0:2, 2:4], X[:, :, 2:4, 0:2].transpose([0, 1, 3, 2])
                )
                # X11 = Aq^-1 - W^T X21
                V.tensor_mul(
                    t7[:], bc(W[:, :, 0, :], 3, sh22), bc(X[:, :, 2, 0:2], 2, sh22)
                )
                V.tensor_mul(
                    t4[:], bc(W[:, :, 1, :], 3, sh22), bc(X[:, :, 3, 0:2], 2, sh22)
                )
                V.tensor_add(t7[:], t7[:], t4[:])
                V.tensor_sub(X[:, :, 0:2, 0:2], Pi[:], t7[:])
                s["X"] = X

            def emit_KG(c):
                s = st[c]
                P12, X = s["P12"], s["X"]
                # term order 2,3,0,1: X rows 2,3 (Sc^-1, X21) are written
                # before rows 0,1, so the PE series can start earlier.
                KG = contract(
                    [
                        (V, bc(P12[:, :, :, n], 3, sh84), bc(X[:, :, n, :], 2, sh84), (8, 4))
                        for n in (2, 3, 0, 1)
                    ],
                    "KG",
                    g * 32,
                    midp,
                )
                nc.sync.dma_start(out=KGv[c], in_=KG[:])

            def emit_all():
                for t in range(nchunk + 4):
                    if t < nchunk:
                        emit_load(t)
                    if 0 <= t - 1 < nchunk:
                        emit_A(t - 1)
                    if 0 <= t - 2 < nchunk:
                        emit_CP(t - 2)
                    if 0 <= t - 4 < nchunk:
                        emit_KG(t - 4)
                    if 0 <= t - 3 < nchunk:
                        emit_SX(t - 3)

            if repeat > 1:
                with tc.For_i(0, repeat, 1):
                    emit_all()
            else:
                emit_all()

    nc.compile()
    return nc


def _get_nc():
    if "nc" not in _NC_CACHE:
        _NC_CACHE["nc"] = _build_nc()
    return _NC_CACHE["nc"]


def kernel(F, H, Sigma_previous, Q, R):
    from concourse.bass_utils import run_bass_kernel_spmd

    nc = _get_nc()
    in_maps = []
    for ci in range(NCORES):
        sl = slice(ci * B_CORE, (ci + 1) * B_CORE)
        in_maps.append(
            {
                "F": np.ascontiguousarray(F[sl], dtype=np.float32),
                "H": np.ascontiguousarray(H[sl], dtype=np.float32),
                "Sigma_previous": np.ascontiguousarray(
                    Sigma_previous[sl], dtype=np.float32
                ),
                "Q": np.ascontiguousarray(Q[sl], dtype=np.float32),
                "R": np.ascontiguousarray(R[sl], dtype=np.float32),
            }
        )
    res = run_bass_kernel_spmd(nc, in_maps, core_ids=list(range(NCORES)))
    return np.concatenate([r["KG"] for r in res.results], axis=0)


# KG term n needs X row n; the (2, 3, 0, 1) order in emit_KG matches the
# order X's rows become available. The n-th PE accumulation term uses the
# same reordering, which is sum-order-irrelevant.



# revision 4
# speedup vs baseline: 1.0101x; 1.0101x over previous
"""Batched Kalman-gain kernel for Trainium2 (Bass/Tile), 8-core data parallel.

Per batch b (262144 of them):
    Sigma = F Sp F^T + Q            [8,8]
    S     = H Sigma H^T + R         [4,4]
    KG    = Sigma H^T S^-1          [8,4]

Factored to avoid materializing Sigma:
    A   = H F                       [4,8]  (stored transposed: At[j,m])
    C   = Sp A^T                    [8,4]
    P12 = F C + Q H^T  (= Sigma H^T) [8,4]
    S   = H P12 + R                 [4,4]
    X   = S^-1  (SPD, 2x2-block Schur complement)
    KG  = P12 X

Mapping: "planes" layout. 128 SBUF partitions = batch lanes; each lane holds
G consecutive batches' matrices along the free axis. Every per-batch product
is a wide elementwise tensor_tensor with broadcast access patterns; all
contraction sums ride the TensorEngine (identity fp32r stationary, PSUM
accumulation across matmuls). ScalarE (ACT) evacuates PSUM->SBUF and seeds
the S accumulation with R.

Measured-AP discipline (this is where the speed comes from): every
tensor_tensor operand keeps its innermost non-broadcast run contiguous —
inner-broadcast (0,n) or contiguous (1,n) inner dims only, never a short
strided run under a broadcast (that class measured ~4x slower). To get
there:
  - H is pre-transposed on the HOST (free) to Ht[b, k, m];
  - A is produced directly in transposed layout At[j, m];
  - Sp and Q column reads use their symmetry (row k == col k);
  - F column reads appear only in the inner-broadcast class (measured fast).

The 4x4 SPD inverse is elementwise via the Schur complement of the leading
2x2 block, with scalar_tensor_tensor fusions for the negations.

Per-chunk stages are software-pipelined with skew across engine queues:
    iter t:  load(t) | A(t-1) | C,P12(t-2) | KG(t-4) | S,inv(t-3)
"""

import numpy as np

P = 128          # SBUF partitions (batch lanes)
G = 32           # consecutive batches per lane per chunk
B = 262144       # full problem batch
NCORES = 8
B_CORE = B // NCORES           # 32768 per core
CHUNK = P * G                  # 4096 batches per chunk
NCHUNK = B_CORE // CHUNK       # 8 chunks

_NC_CACHE = {}


def _build_nc(b_core=B_CORE, g=G):
    import concourse.bacc as bacc
    import concourse.mybir as mybir
    import concourse.tile as tile
    from concourse.masks import make_identity

    fp32 = mybir.dt.float32
    fp32r = mybir.dt.float32r
    MULT = mybir.AluOpType.mult
    SUB = mybir.AluOpType.subtract
    ADD = mybir.AluOpType.add

    nchunk = b_core // (P * g)
    assert nchunk * P * g == b_core
    nc = bacc.Bacc("TRN2", target_bir_lowering=False, debug=False)

    F_d = nc.dram_tensor("F", [b_core, 8, 8], fp32, kind="ExternalInput").ap()
    Ht_d = nc.dram_tensor("Ht", [b_core, 8, 4], fp32, kind="ExternalInput").ap()
    Sp_d = nc.dram_tensor(
        "Sigma_previous", [b_core, 8, 8], fp32, kind="ExternalInput"
    ).ap()
    Q_d = nc.dram_tensor("Q", [b_core, 8, 8], fp32, kind="ExternalInput").ap()
    R_d = nc.dram_tensor("R", [b_core, 4, 4], fp32, kind="ExternalInput").ap()
    KG_d = nc.dram_tensor("KG", [b_core, 8, 4], fp32, kind="ExternalOutput").ap()

    # chunk views: batch = c*(P*g) + p*g + g_idx  (lane-contiguous DMA)
    Fv = F_d.rearrange("(c p g) i j -> c p g i j", p=P, g=g)
    Htv = Ht_d.rearrange("(c p g) k m -> c p g k m", p=P, g=g)
    Spv = Sp_d.rearrange("(c p g) i j -> c p g i j", p=P, g=g)
    Qv = Q_d.rearrange("(c p g) i j -> c p g i j", p=P, g=g)
    Rv = R_d.rearrange("(c p g) m n -> c p g m n", p=P, g=g)
    KGv = KG_d.rearrange("(c p g) i m -> c p g i m", p=P, g=g)

    BANK = 512  # fp32 elems per PSUM bank per partition

    with tile.TileContext(nc) as tc:
        with (
            tc.tile_pool(name="consts", bufs=1) as consts,
            tc.tile_pool(name="ins3", bufs=3) as insp,
            tc.tile_pool(name="ins4", bufs=4) as insp2,
            tc.tile_pool(name="mid2", bufs=2) as midp,
            tc.tile_pool(name="mid3", bufs=3) as midp3,
            tc.tile_pool(name="prod", bufs=4) as prodp,
            tc.tile_pool(name="inv", bufs=2) as invp,
            tc.tile_pool(name="psum", bufs=8, space="PSUM") as psump,
        ):
            ident = consts.tile([P, P], fp32, tag="ident")
            make_identity(nc, ident[:])
            identr_t = consts.tile([P, P], fp32r, tag="identr")
            nc.vector.tensor_copy(identr_t[:], ident[:])
            identr = identr_t[:]

            V = nc.vector
            GP = nc.gpsimd
            SC = nc.scalar

            def flat(t):
                return t[:].rearrange("p g a b -> p (g a b)")

            def contract(terms, out_tag, width, pool, seed=None):
                """terms: list of (engine, in0_ap, in1_ap, prod_shape[2:]).
                Returns SBUF tile [P, g, a, b] = sum of products (+seed).
                Products are elementwise TT ops; the sum runs on the PE via
                fp32r identity matmuls accumulating in PSUM. `seed` (an AP
                of `width` elems) is copied into PSUM by ACT first and the
                matmuls accumulate onto it."""
                npc = (width + BANK - 1) // BANK
                d0, d1 = terms[0][3]
                out = pool.tile([P, g, d0, d1], fp32, tag=out_tag, name=out_tag)
                outf = flat(out)
                ps_tiles = [
                    psump.tile([P, BANK], fp32, tag="ps", name=f"ps_{out_tag}_{pc}")
                    for pc in range(npc)
                ]
                if seed is not None:
                    for pc in range(npc):
                        lo, hi = pc * BANK, min((pc + 1) * BANK, width)
                        SC.copy(ps_tiles[pc][:, : hi - lo], seed[:, lo:hi])
                rhs_list = []
                for eng, a_ap, b_ap, (e0, e1) in terms:
                    etag = "pv" if eng is V else "pg"
                    pt = prodp.tile([P, g, e0, e1], fp32r, tag=etag, name=etag)
                    eng.tensor_tensor(pt[:], a_ap, b_ap, op=MULT)
                    rhs_list.append(flat(pt))
                nterm = len(rhs_list)
                for pc in range(npc):
                    lo, hi = pc * BANK, min((pc + 1) * BANK, width)
                    for t, rhs in enumerate(rhs_list):
                        nc.tensor.matmul(
                            ps_tiles[pc][:, : hi - lo],
                            identr,
                            rhs[:, lo:hi],
                            start=(t == 0 and seed is None),
                            stop=(t == nterm - 1),
                            skip_group_check=(seed is not None),
                        )
                    SC.copy(outf[:, lo:hi], ps_tiles[pc][:, : hi - lo])
                return out

            def bc(ap, axis, shape):
                return ap.unsqueeze(axis).broadcast_to(shape)

            st = [dict() for _ in range(nchunk)]
            sh84 = [P, g, 8, 4]
            sh44 = [P, g, 4, 4]
            sh22 = [P, g, 2, 2]

            def emit_load(c):
                s = st[c]
                s["F"] = insp.tile([P, g, 8, 8], fp32, tag="F", name="Ft")
                s["Sp"] = insp.tile([P, g, 8, 8], fp32, tag="Sp", name="Spt")
                s["Q"] = insp.tile([P, g, 8, 8], fp32, tag="Q", name="Qt")
                s["Ht"] = insp2.tile([P, g, 8, 4], fp32, tag="Ht", name="Htt")
                s["R"] = insp2.tile([P, g, 4, 4], fp32, tag="R", name="Rt")
                nc.sync.dma_start(out=s["F"][:], in_=Fv[c])
                nc.sync.dma_start(out=s["Ht"][:], in_=Htv[c])
                nc.sync.dma_start(out=s["Sp"][:], in_=Spv[c])
                nc.sync.dma_start(out=s["Q"][:], in_=Qv[c])
                nc.sync.dma_start(out=s["R"][:], in_=Rv[c])

            def emit_A(c):
                # At[j,m] = sum_k F[k,j] * Ht[k,m]
                s = st[c]
                Ft, Htt = s["F"], s["Ht"]
                s["At"] = contract(
                    [
                        (V, bc(Ft[:, :, k, :], 3, sh84), bc(Htt[:, :, k, :], 2, sh84), (8, 4))
                        for k in range(8)
                    ],
                    "At",
                    g * 32,
                    midp,
                )

            def emit_CP(c):
                # C[i,m] = sum_k Sp[k,i] * At[k,m]   (Sp symmetric: row k = col k)
                s = st[c]
                Ft, Spt, Qt, Htt, At = s["F"], s["Sp"], s["Q"], s["Ht"], s["At"]
                C = contract(
                    [
                        (V, bc(Spt[:, :, k, :], 3, sh84), bc(At[:, :, k, :], 2, sh84), (8, 4))
                        for k in range(8)
                    ],
                    "C",
                    g * 32,
                    midp,
                )
                # P12[i,m] = sum_j F[i,j] C[j,m] + sum_j Q[j,i] Ht[j,m]
                # (Q symmetric). Q terms first: independent of C, fill the
                # engine while C is still accumulating/evacuating.
                s["P12"] = contract(
                    [
                        (V, bc(Qt[:, :, j, :], 3, sh84), bc(Htt[:, :, j, :], 2, sh84), (8, 4))
                        for j in range(8)
                    ]
                    + [
                        (V, bc(Ft[:, :, :, j], 3, sh84), bc(C[:, :, j, :], 2, sh84), (8, 4))
                        for j in range(8)
                    ],
                    "P12",
                    g * 32,
                    midp3,
                )

            def emit_SX(c):
                # S[m,n] = sum_i Ht[i,m] * P12[i,n] + R   (R seeded via ACT)
                s = st[c]
                Htt, Rt, P12 = s["Ht"], s["R"], s["P12"]
                S = contract(
                    [
                        (GP, bc(Htt[:, :, i, :], 3, sh44), bc(P12[:, :, i, :], 2, sh44), (4, 4))
                        for i in range(8)
                    ],
                    "S",
                    g * 16,
                    midp,
                    seed=flat(Rt),
                )
                # ---- X = S^-1 via Schur complement of leading 2x2 block ----
                X = midp.tile([P, g, 4, 4], fp32, tag="X", name="X")
                Pi = invp.tile([P, g, 2, 2], fp32, tag="Pi", name="Pi")
                W = invp.tile([P, g, 2, 2], fp32, tag="W", name="W")
                Sc = invp.tile([P, g, 2, 2], fp32, tag="Sc", name="Sc")
                t3 = invp.tile([P, g, 2, 2], fp32, tag="t3", name="t3")
                t4 = invp.tile([P, g, 2, 2], fp32, tag="t4", name="t4")
                u0 = invp.tile([P, g], fp32, tag="u0", name="u0")
                u1 = invp.tile([P, g], fp32, tag="u1", name="u1")
                d0 = invp.tile([P, g], fp32, tag="d0", name="d0")
                r0 = invp.tile([P, g], fp32, tag="r0", name="r0")
                d1 = invp.tile([P, g], fp32, tag="d1", name="d1")
                r1 = invp.tile([P, g], fp32, tag="r1", name="r1")

                sa = S[:, :, 0, 0]
                sb = S[:, :, 0, 1]
                sc_ = S[:, :, 1, 1]
                Bq = S[:, :, 2:4, 0:2]  # lower-left 2x2 block
                Dq = S[:, :, 2:4, 2:4]
                Pif = Pi[:].rearrange("p g a b -> p g (a b)")

                # Pi = Aq^-1 (2x2 symmetric inverse)
                V.tensor_mul(u0[:], sa, sc_)
                V.tensor_mul(u1[:], sb, sb)
                V.tensor_tensor(d0[:], u0[:], u1[:], op=SUB)
                V.reciprocal(r0[:], d0[:])
                V.tensor_mul(Pi[:, :, 0, 0], sc_, r0[:])
                V.tensor_mul(Pi[:, :, 1, 1], sa, r0[:])
                # Pi[0,1] = Pi[1,0] = -sb*r0 in one fused op (contig elems 1:3)
                V.scalar_tensor_tensor(
                    Pif[:, :, 1:3],
                    bc(sb, 2, [P, g, 2]),
                    -1.0,
                    bc(r0[:], 2, [P, g, 2]),
                    op0=MULT,
                    op1=MULT,
                )
                # W = Bq @ Pi
                V.tensor_mul(t3[:], bc(Bq[:, :, :, 0], 3, sh22), bc(Pi[:, :, 0, :], 2, sh22))
                V.tensor_mul(t4[:], bc(Bq[:, :, :, 1], 3, sh22), bc(Pi[:, :, 1, :], 2, sh22))
                V.tensor_tensor(W[:], t3[:], t4[:], op=ADD)
                # Sc = Dq - W Bq^T
                V.tensor_mul(t3[:], bc(W[:, :, :, 0], 3, sh22), bc(Bq[:, :, :, 0], 2, sh22))
                V.tensor_mul(t4[:], bc(W[:, :, :, 1], 3, sh22), bc(Bq[:, :, :, 1], 2, sh22))
                V.tensor_tensor(t3[:], t3[:], t4[:], op=ADD)
                V.tensor_tensor(Sc[:], Dq, t3[:], op=SUB)
                # Sc^-1 -> X[2:4,2:4]
                V.tensor_mul(u0[:], Sc[:, :, 0, 0], Sc[:, :, 1, 1])
                V.tensor_mul(u1[:], Sc[:, :, 0, 1], Sc[:, :, 0, 1])
                V.tensor_tensor(d1[:], u0[:], u1[:], op=SUB)
                V.reciprocal(r1[:], d1[:])
                V.tensor_mul(X[:, :, 2, 2], Sc[:, :, 1, 1], r1[:])
                V.tensor_mul(X[:, :, 3, 3], Sc[:, :, 0, 0], r1[:])
                V.scalar_tensor_tensor(
                    X[:, :, 2, 3], Sc[:, :, 0, 1], -1.0, r1[:], op0=MULT, op1=MULT
                )
                V.tensor_copy(X[:, :, 3, 2], X[:, :, 2, 3])
                # X21 = -Sc^-1 W -> X[2:4,0:2]
                V.tensor_mul(t3[:], bc(X[:, :, 2:4, 2], 3, sh22), bc(W[:, :, 0, :], 2, sh22))
                V.tensor_mul(t4[:], bc(X[:, :, 2:4, 3], 3, sh22), bc(W[:, :, 1, :], 2, sh22))
                V.tensor_tensor(t3[:], t3[:], t4[:], op=ADD)
                V.tensor_scalar_mul(X[:, :, 2:4, 0:2], t3[:], -1.0)
                # X12 = X21^T
                V.tensor_copy(
                    X[:, :, 0:2, 2:4], X[:, :, 2:4, 0:2].transpose([0, 1, 3, 2])
                )
                # X11 = Pi - W^T X21
                V.tensor_mul(t3[:], bc(W[:, :, 0, :], 3, sh22), bc(X[:, :, 2, 0:2], 2, sh22))
                V.tensor_mul(t4[:], bc(W[:, :, 1, :], 3, sh22), bc(X[:, :, 3, 0:2], 2, sh22))
                V.tensor_tensor(t3[:], t3[:], t4[:], op=ADD)
                V.tensor_tensor(X[:, :, 0:2, 0:2], Pi[:], t3[:], op=SUB)
                s["X"] = X

            def emit_KG(c):
                # KG[i,m] = sum_n P12[i,n] X[n,m]; term order 2,3,0,1 matches
                # the order X's rows become available. S/KG products ride the
                # Pool engine to offload the DVE.
                s = st[c]
                P12, X = s["P12"], s["X"]
                KG = contract(
                    [
                        (GP, bc(P12[:, :, :, n], 3, sh84), bc(X[:, :, n, :], 2, sh84), (8, 4))
                        for n in (2, 3, 0, 1)
                    ],
                    "KG",
                    g * 32,
                    midp,
                )
                nc.sync.dma_start(out=KGv[c], in_=KG[:])

            for t in range(nchunk + 4):
                if t < nchunk:
                    emit_load(t)
                if 0 <= t - 1 < nchunk:
                    emit_A(t - 1)
                if 0 <= t - 2 < nchunk:
                    emit_CP(t - 2)
                if 0 <= t - 4 < nchunk:
                    emit_KG(t - 4)
                if 0 <= t - 3 < nchunk:
                    emit_SX(t - 3)

    nc.compile()
    return nc


def _get_nc():
    if "nc" not in _NC_CACHE:
        _NC_CACHE["nc"] = _build_nc()
    return _NC_CACHE["nc"]


def make_in_maps(F, H, Sigma_previous, Q, R):
    in_maps = []
    for ci in range(NCORES):
        sl = slice(ci * B_CORE, (ci + 1) * B_CORE)
        in_maps.append(
            {
                "F": np.ascontiguousarray(F[sl], dtype=np.float32),
                "Ht": np.ascontiguousarray(
                    H[sl].transpose(0, 2, 1), dtype=np.float32
                ),
                "Sigma_previous": np.ascontiguousarray(
                    Sigma_previous[sl], dtype=np.float32
                ),
                "Q": np.ascontiguousarray(Q[sl], dtype=np.float32),
                "R": np.ascontiguousarray(R[sl], dtype=np.float32),
            }
        )
    return in_maps


def kernel(F, H, Sigma_previous, Q, R):
    from concourse.bass_utils import run_bass_kernel_spmd

    nc = _get_nc()
    in_maps = make_in_maps(F, H, Sigma_previous, Q, R)
    res = run_bass_kernel_spmd(nc, in_maps, core_ids=list(range(NCORES)))
    return np.concatenate([r["KG"] for r in res.results], axis=0)
